# revision 1
# baseline (speedup 1.0000x reference)
"""Soft decision-tree forward (nn_DTree) on 8 trn2 NeuronCores.

Strategy (pure data parallel, per the sharding hint):
  - shard x row-wise 8 ways; replicate the tiny tree params.
  - per core: z = [x|1|1] @ [W | -c_hi | -c_lo]^T via bf16 PE matmuls into
    fp32 PSUM, g = sigmoid(z) on ACT, then a level-by-level value-tree
    blend on DVE:
       V_k = g_k * (V_{k+1,L} - V_{k+1,R}) + V_{k+1,R}
    with nodes pre-permuted (level-major, left-children-first) so every
    level's children are two contiguous halves of the previous level.
  - x reaches the PE transposed via the DMA xbar: x is cast to bf16 into a
    [rows, 64] DRAM bounce (cols 0-31 = features, 32-33 = bias-ones slots),
    whose [rows/2, 128] view is xbar-transpose-legal (cols % 128 == 0).
    The transposed SBUF buffer holds two interleaved row-classes
    (rows 2j+s at partitions 64s+f); the resulting row permutation of the
    [128, tiles] output is undone on the host (128 KiB reorder).
"""

import numpy as np
import ml_dtypes

import concourse.bass as bass
import concourse.bacc as bacc
import concourse.tile as tile
from concourse import mybir
from concourse.bass_utils import run_bass_kernel_spmd

BF16 = ml_dtypes.bfloat16

F = 32
D = 8
NODES = 255
LEAVES = 256
N_FULL = 262144
N_CORES = 8
ROWS = N_FULL // N_CORES  # 32768 rows per core

# level-major offsets of each level's gates inside the 255-column block
LEVEL_OFF = {7: 0, 6: 128, 5: 192, 4: 224, 3: 240, 2: 248, 1: 252, 0: 254}


def _orderings():
    """ord[k] = local node order at level k (left-children-first recursion)."""
    ordv = {0: [0]}
    for k in range(7):
        ordv[k + 1] = [2 * i for i in ordv[k]] + [2 * i + 1 for i in ordv[k]]
    col_nodes = []
    for k in range(7, -1, -1):
        base = 2 ** k - 1
        col_nodes += [base + i for i in ordv[k]]
    return ordv, np.array(col_nodes)


def host_prep(feature_importances, feature_splits, leaf_node_classes, slots):
    """Tiny-param preprocessing (O(8K) work): relu/sigmoid/c, node permutation,
    bf16 weight matrix with split bias rows, leaf-blend constants."""
    fi = np.asarray(feature_importances, np.float32).reshape(NODES, F)
    fs = np.asarray(feature_splits, np.float32).reshape(NODES, F)
    cls = np.asarray(leaf_node_classes, np.float32).reshape(LEAVES)

    W = np.maximum(fi, 0.0)
    S = 1.0 / (1.0 + np.exp(-fs))
    c = np.sum(W * S, axis=1)  # (NODES,)

    ordv, col_nodes = _orderings()
    Wp = W[col_nodes]          # (255, 32) permuted level-major
    cp = c[col_nodes]

    c_hi = cp.astype(BF16).astype(np.float32)
    c_lo = (cp - c_hi).astype(np.float32)

    wt = np.zeros((128, 256), BF16)
    for b in (0, 64):  # replicate for both row-class partition groups
        wt[b : b + F, 0:NODES] = Wp.T.astype(BF16)
        wt[b + F, 0:NODES] = (-c_hi).astype(BF16)
        wt[b + F + 1, 0:NODES] = (-c_lo).astype(BF16)

    o7 = np.array(ordv[7])
    delta = (cls[2 * o7] - cls[2 * o7 + 1]).astype(BF16)
    beta = cls[2 * o7 + 1].astype(BF16)
    # (node, slot) layout: value for node j replicated across `slots` columns
    db = np.zeros((128, 2 * slots * 128), BF16)
    db[:, : slots * 128] = np.repeat(delta, slots)[None, :]
    db[:, slots * 128 :] = np.repeat(beta, slots)[None, :]
    return wt, db


def out_permutation(rows, chunk):
    """Physical row index for each (partition p, device-output column) pair.

    Device tiles are emitted per (chunk ci, class s, tile t); the 128 rows of
    that tile are {ci*chunk + 2*(128*t + p) + s}.
    """
    tiles = rows // 128
    tpc = chunk // 256  # tiles per class within one chunk
    perm = np.empty((128, tiles), np.int64)
    col = 0
    for ci in range(rows // chunk):
        for s in range(2):
            for t in range(tpc):
                p = np.arange(128)
                perm[:, col] = ci * chunk + 2 * (128 * t + p) + s
                col += 1
    return perm


def build_nc(rows, slots, chunk, repeat=1, stage=4, mm_base0=False, act_flat=False,
             tiny=False, gbufs=2, pw_opt=8, tchunk=None, osplit=1):
    """Build the single-core Bass program (SPMD across the cores).

    repeat>1 re-runs the whole compute pipeline (for timing: the wall-clock
    delta between repeat=K and repeat=1 isolates on-device kernel time from
    host/transfer overhead).
    """
    assert rows % 128 == 0 and chunk % 256 == 0 and rows % chunk == 0
    tiles = rows // 128
    assert tiles % slots == 0
    groups = tiles // slots
    assert (chunk // 256) % 1 == 0
    bf = mybir.dt.bfloat16
    f32 = mybir.dt.float32

    nc = bacc.Bacc()
    x_in = nc.dram_tensor("x", [rows, F], f32, kind="ExternalInput")
    wt_in = nc.dram_tensor("wt", [128, 256], bf, kind="ExternalInput")
    db_in = nc.dram_tensor("db", [128, 2 * slots * 128], bf, kind="ExternalInput")
    ones_in = nc.dram_tensor("ones", [2, rows // 2], bf, kind="ExternalInput")
    out_dram = nc.dram_tensor("out", [128, tiles], f32, kind="ExternalOutput")

    n_chunks = rows // chunk
    tpc = chunk // 256  # tiles per class per chunk

    with tile.TileContext(nc) as tc:
        with (
            tc.tile_pool(name="consts", bufs=1) as consts,
            tc.tile_pool(name="xT", bufs=1) as xtp,
            tc.tile_pool(name="dram", bufs=1, space="DRAM") as dram,
            tc.tile_pool(name="zps", bufs=2 if pw_opt <= 8 else 1, space="PSUM") as zps,
            tc.tile_pool(name="gpool", bufs=gbufs) as gpool,
            tc.tile_pool(name="vpool", bufs=2 if slots <= 32 else 1) as vpool,
            tc.tile_pool(name="dpool", bufs=2 if slots <= 32 else 1) as dpool,
            tc.tile_pool(name="opool", bufs=1) as opool,
        ):
            # ---- constants ----
            wt_sb = consts.tile([128, 256], bf)
            nc.sync.dma_start(out=wt_sb[:], in_=wt_in[:])
            dbt = consts.tile([128, 2 * slots * 128], bf)
            nc.sync.dma_start(out=dbt[:], in_=db_in[:])
            dbc = dbt[:, 0 : slots * 128]
            bbc = dbt[:, slots * 128 :]

            # ---- x: cast bf16 -> [rows, 64] DRAM bounce, xbar transpose ----
            # xT2 chunk view: partition 64*s + f, col j  ==  x[2j+s, f]
            for _rep in range(repeat):
              xT2 = xtp.tile([128, rows // 2], bf, tag="xT2")
              xbf = dram.tile([rows, 64], bf, tag="xbf")
              tck = tchunk or chunk
              for ci in range(n_chunks):
                  sl = slice(ci * chunk, (ci + 1) * chunk)
                  nc.gpsimd.dma_start(out=xbf[sl, 0:F], in_=x_in[sl, :])
              if stage >= 2:
                  for ci in range(rows // tck):
                      sl = slice(ci * tck, (ci + 1) * tck)
                      sl2 = slice(ci * tck // 2, (ci + 1) * tck // 2)
                      src = xbf[sl, :].rearrange("(r two) c -> r (two c)", two=2)
                      nc.sync.dma_start_transpose(out=xT2[:, sl2], in_=src)
                      # bias-ones rows (overwrite transposed junk)
                      nc.sync.dma_start(out=xT2[32:34, sl2], in_=ones_in[:, sl2])
                      nc.sync.dma_start(out=xT2[96:98, sl2], in_=ones_in[:, sl2])

              out_sb = opool.tile([128, tiles], f32)
              if stage == 1:
                  nc.gpsimd.dma_start(out=out_dram[:, 0:1], in_=xbf[0:128, 0:1])
                  continue
              if stage == 2:
                  nc.gpsimd.dma_start(out=out_dram[:, 0:1], in_=xT2[:, 0:1])
                  continue

              # global tile g -> (lhsT slice of xT2, matching-base rhs slice)
              def operands_of(g):
                  ci, r = divmod(g, 2 * tpc)
                  s, t = divmod(r, tpc)
                  c0 = ci * chunk // 2 + t * 128
                  b = 0 if mm_base0 else 64 * s
                  return (
                      xT2[b : b + 34, c0 : c0 + 128],
                      wt_sb[b : b + 34, 0:NODES],
                  )

              pw = min(pw_opt, slots)  # tiles per PSUM wave
              for gi in range(groups):
                  # g layout: [128, node, slot] — every tree slice is a flat
                  # contiguous range, keeping DVE in the bf16 2x perf mode.
                  g_t = gpool.tile([128, 256, slots], bf)
                  nn = 32 if tiny else NODES
                  for half in range(slots // pw):
                      zt = zps.tile([128, pw * 256], f32)
                      ztv = zt[:].rearrange("p (j c) -> p c j", c=256)
                      for j in range(pw):
                          g = gi * slots + half * pw + j
                          lhs, rhs = operands_of(g)
                          nc.tensor.matmul(
                              ztv[:, 0:nn, j],
                              lhsT=lhs,
                              rhs=rhs[:, 0:nn],
                              start=True,
                              stop=True,
                          )
                      if act_flat:
                          aw = pw * (32 if tiny else 256)
                          nc.scalar.activation(
                              out=g_t[:].rearrange("p a b -> p (a b)")[
                                  :, half * aw : (half + 1) * aw
                              ],
                              in_=zt[:, 0:aw],
                              func=mybir.ActivationFunctionType.Sigmoid,
                          )
                      else:
                          nc.scalar.activation(
                              out=g_t[:, 0:NODES, half * pw : (half + 1) * pw],
                              in_=ztv[:, 0:NODES, :],
                              func=mybir.ActivationFunctionType.Sigmoid,
                          )
                  if stage == 3:
                      nc.vector.tensor_copy(
                          out_sb[:, gi * slots : (gi + 1) * slots],
                          g_t[:, 0, :],
                      )
                      continue
                  # ---- value tree ----
                  g_flat = g_t[:].rearrange("p a b -> p (a b)")
                  v = vpool.tile([128, 128 * slots], bf, tag="v7")
                  nc.vector.tensor_mul(v[:], g_flat[:, 0 : 128 * slots], dbc)
                  nc.vector.tensor_add(v[:], v[:], bbc)
                  for k in range(6, -1, -1):
                      m = 2 ** k
                      off = LEVEL_OFF[k]
                      vl = v[:, 0 : m * slots]
                      vr = v[:, m * slots : 2 * m * slots]
                      dt = dpool.tile([128, m * slots], bf, tag="dtmp")
                      nc.vector.tensor_sub(dt[:], vl, vr)
                      gk = g_flat[:, off * slots : (off + m) * slots]
                      if k > 0:
                          vn = vpool.tile([128, m * slots], bf, tag=f"v{k}")
                          nc.vector.tensor_mul(vn[:], gk, dt[:])
                          nc.vector.tensor_add(vn[:], vn[:], vr)
                          v = vn
                      else:
                          vo = out_sb[:, gi * slots : (gi + 1) * slots]
                          nc.vector.tensor_mul(vo, gk, dt[:])
                          nc.vector.tensor_add(vo, vo, vr)

              step = max(1, groups // osplit)
              for g0 in range(0, groups, step):
                  c0, c1 = g0 * slots, min((g0 + step) * slots, tiles)
                  nc.sync.dma_start(out=out_dram[:, c0:c1], in_=out_sb[:, c0:c1])
    return nc


_CACHE = {}


def _get_nc(rows, slots, chunk):
    key = (rows, slots, chunk)
    if key not in _CACHE:
        nc = build_nc(rows, slots, chunk, gbufs=4, osplit=4)
        if not nc.is_finalized():
            nc.finalize()
        _CACHE[key] = nc
    return _CACHE[key]


def run_device(x, wt, db, slots, chunk, n_cores=N_CORES, trace=False):
    rows = x.shape[0] // n_cores
    nc = _get_nc(rows, slots, chunk)
    ones1 = np.ones((2, rows // 2), BF16)
    in_maps = [
        {
            "x": np.ascontiguousarray(x[i * rows : (i + 1) * rows]),
            "wt": wt,
            "db": db,
            "ones": ones1,
        }
        for i in range(n_cores)
    ]
    res = run_bass_kernel_spmd(nc, in_maps, list(range(n_cores)), trace=trace)
    perm = out_permutation(rows, chunk)
    out = np.empty((n_cores * rows, 1), np.float32)
    for i in range(n_cores):
        dev = res.results[i]["out"].astype(np.float32)  # [128, tiles]
        core_out = np.empty(rows, np.float32)
        core_out[perm.ravel()] = dev.ravel()
        out[i * rows : (i + 1) * rows, 0] = core_out
    return out, res


def kernel(**inputs):
    x = np.asarray(inputs["x"], np.float32).reshape(-1, F)
    slots, chunk = 32, 4096
    wt, db = host_prep(
        inputs["feature_importances"],
        inputs["feature_splits"],
        inputs["leaf_node_classes"],
        slots,
    )
    out, _ = run_device(x, wt, db, slots, chunk)
    return out



# revision 20
# speedup vs baseline: 1.6238x; 1.6238x over previous
"""Soft decision-tree forward (nn_DTree) on 8 trn2 NeuronCores.

Strategy (pure data parallel, per the sharding hint):
  - shard x row-wise 8 ways; replicate the tiny tree params.
  - per core: z = [x|1|1] @ [W | -c_hi | -c_lo]^T via bf16 PE matmuls into
    fp32 PSUM, g = sigmoid(z) on ACT, then a level-by-level value-tree
    blend:
       V_k = g_k * (V_{k+1,L} - V_{k+1,R}) + V_{k+1,R}
    with nodes pre-permuted (level-major, left-children-first) so every
    level's children are two contiguous halves of the previous level.
  - the blend is split across the DVE and the (otherwise idle) GPSIMD
    engine at a ~5:3 group ratio so neither exceeds the ACT sigmoid time;
    the two widest blend levels are sliced per sigmoid wave to hide the
    pipeline tail.
  - x reaches the PE transposed via the DMA xbar: x is cast to bf16 into a
    [rows, 64] DRAM bounce (cols 0-31 = features, 32-33 = bias ones),
    whose [rows/2, 128] view is xbar-transpose-legal (cols % 128 == 0).
    The bias-ones land on partitions 32:34 / 96:98 via the transpose
    itself (cols 32:34 of even/odd rows), so no per-chunk fixup DMAs.
    The transposed SBUF buffer holds two interleaved row-classes
    (rows 2j+s at partitions 64s+f); the resulting row permutation of the
    [128, tiles] output is undone on the host (128 KiB reorder).
"""

import numpy as np
import ml_dtypes

import concourse.bass as bass
import concourse.bacc as bacc
import concourse.tile as tile
from concourse import mybir
from concourse.bass_utils import run_bass_kernel_spmd

BF16 = ml_dtypes.bfloat16

F = 32
D = 8
NODES = 255
LEAVES = 256
N_FULL = 262144
N_CORES = 8
ROWS = N_FULL // N_CORES  # 32768 rows per core

# level-major offsets of each level's gates inside the 255-column block
LEVEL_OFF = {7: 0, 6: 128, 5: 192, 4: 224, 3: 240, 2: 248, 1: 252, 0: 254}


def _orderings():
    """ord[k] = local node order at level k (left-children-first recursion)."""
    ordv = {0: [0]}
    for k in range(7):
        ordv[k + 1] = [2 * i for i in ordv[k]] + [2 * i + 1 for i in ordv[k]]
    col_nodes = []
    for k in range(7, -1, -1):
        base = 2 ** k - 1
        col_nodes += [base + i for i in ordv[k]]
    return ordv, np.array(col_nodes)


def host_prep(feature_importances, feature_splits, leaf_node_classes, slots):
    """Tiny-param preprocessing (O(8K) work): relu/sigmoid/c, node permutation,
    bf16 weight matrix with split bias rows, leaf-blend constants."""
    fi = np.asarray(feature_importances, np.float32).reshape(NODES, F)
    fs = np.asarray(feature_splits, np.float32).reshape(NODES, F)
    cls = np.asarray(leaf_node_classes, np.float32).reshape(LEAVES)

    W = np.maximum(fi, 0.0)
    S = 1.0 / (1.0 + np.exp(-fs))
    c = np.sum(W * S, axis=1)  # (NODES,)

    ordv, col_nodes = _orderings()
    Wp = W[col_nodes]          # (255, 32) permuted level-major
    cp = c[col_nodes]

    c_hi = cp.astype(BF16).astype(np.float32)
    c_lo = (cp - c_hi).astype(np.float32)

    wt = np.zeros((128, 256), BF16)
    for b in (0, 64):  # replicate for both row-class partition groups
        wt[b : b + F, 0:NODES] = Wp.T.astype(BF16)
        wt[b + F, 0:NODES] = (-c_hi).astype(BF16)
        wt[b + F + 1, 0:NODES] = (-c_lo).astype(BF16)

    o7 = np.array(ordv[7])
    delta = (cls[2 * o7] - cls[2 * o7 + 1]).astype(BF16)
    beta = cls[2 * o7 + 1].astype(BF16)
    # (node, slot) layout replicated across `slots` columns; every sigmoid
    # wave reads the same slots-wide window, so only one wave's worth is kept
    db = np.zeros((128, 2 * slots * 128), BF16)
    db[:, : slots * 128] = np.repeat(delta, slots)[None, :]
    db[:, slots * 128 :] = np.repeat(beta, slots)[None, :]
    return wt, db


def out_permutation(rows, chunk):
    """Physical row index for each (partition p, device-output column) pair.

    Device tiles are emitted per (chunk ci, class s, tile t); the 128 rows of
    that tile are {ci*chunk + 2*(128*t + p) + s}.
    """
    tiles = rows // 128
    tpc = chunk // 256  # tiles per class within one chunk
    perm = np.empty((128, tiles), np.int64)
    col = 0
    for ci in range(rows // chunk):
        for s in range(2):
            for t in range(tpc):
                p = np.arange(128)
                perm[:, col] = ci * chunk + 2 * (128 * t + p) + s
                col += 1
    return perm


def build_nc(rows, slots, chunk, repeat=1, gbufs=4, pw_opt=8, osplit=8,
             split_last=True):
    """Build the single-core Bass program (SPMD across the cores).

    The blend is split between DVE and GPSIMD (Pool) at wave granularity by
    a greedy queued-time balance (DVE ~0.52 ns/elem in the 2x mode, Pool
    ~0.83), so both engines stay fed from every sigmoid wave. The two widest
    blend levels (leaf init + k=6) are emitted per sigmoid wave (pw slots)
    so they overlap ACT; the final group's k<=5 chain is split by slot
    halves across both engines to shorten the pipeline tail.
    """
    assert rows % 128 == 0 and chunk % 256 == 0 and rows % chunk == 0
    tiles = rows // 128
    assert tiles % slots == 0
    groups = tiles // slots
    bf = mybir.dt.bfloat16
    f32 = mybir.dt.float32
    pw = min(pw_opt, slots)  # tiles per PSUM wave
    waves = slots // pw

    nc = bacc.Bacc()
    xt_in = nc.dram_tensor("xt", [128, rows // 2], bf, kind="ExternalInput")
    wt_in = nc.dram_tensor("wt", [128, 256], bf, kind="ExternalInput")
    db_in = nc.dram_tensor("db", [128, 2 * pw * 128], bf, kind="ExternalInput")
    out_dram = nc.dram_tensor("out", [128, tiles], f32, kind="ExternalOutput")

    n_chunks = rows // chunk
    tpc = chunk // 256  # tiles per class per chunk

    with tile.TileContext(nc) as tc:
        with (
            tc.tile_pool(name="consts", bufs=1) as consts,
            tc.tile_pool(name="xT", bufs=1) as xtp,
            tc.tile_pool(name="dram", bufs=1, space="DRAM") as dram,
            tc.tile_pool(name="zps", bufs=2, space="PSUM") as zps,
            tc.tile_pool(name="gpool", bufs=gbufs) as gpool,
            tc.tile_pool(name="vpool", bufs=2) as vpool,
            tc.tile_pool(name="dpool", bufs=2) as dpool,
            tc.tile_pool(name="vpoolp", bufs=2) as vpoolp,
            tc.tile_pool(name="dpoolp", bufs=2) as dpoolp,
            tc.tile_pool(name="opool", bufs=1) as opool,
        ):
            # ---- constants ----
            wt_sb = consts.tile([128, 256], bf)
            nc.sync.dma_start(out=wt_sb[:], in_=wt_in[:])
            # warm the sigmoid activation table during the DMA prologue so the
            # first real activation doesn't pay the table load
            warm = consts.tile([1, 8], f32)
            nc.scalar.activation(out=warm[:], in_=wt_sb[0:1, 0:8],
                                 func=mybir.ActivationFunctionType.Sigmoid)
            # dbt loads on the idle Pool queue, ready before the first blend
            dbt = consts.tile([128, 2 * pw * 128], bf)
            half_db = pw * 128
            nc.gpsimd.dma_start(out=dbt[:, 0:half_db], in_=db_in[:, 0:half_db])
            nc.gpsimd.dma_start(out=dbt[:, half_db:], in_=db_in[:, half_db:])

            # ---- x arrives host-transposed: partition 64*s + f, col j ==
            # x[2j+s, f]; bias-ones pre-filled on partitions 32:34 / 96:98.
            # Chunked DMA so the first matmul wave starts ~4us in.
            for _rep in range(repeat):
              xT2 = xtp.tile([128, rows // 2], bf, tag="xT2")
              half_cols = rows // 2
              xsl = [(0, 1024)] + [
                  (1024 + i * 2048, min(1024 + (i + 1) * 2048, half_cols))
                  for i in range((half_cols - 1024 + 2047) // 2048)
              ]
              for c0, c1 in xsl:
                  nc.sync.dma_start(out=xT2[:, c0:c1], in_=xt_in[:, c0:c1])

              out_sb = opool.tile([128, tiles], f32)

              # global tile g -> (lhsT slice of xT2, matching-base rhs slice)
              def operands_of(g):
                  ci, r = divmod(g, 2 * tpc)
                  s, t = divmod(r, tpc)
                  c0 = ci * chunk // 2 + t * 128
                  b = 64 * s
                  return (
                      xT2[b : b + 34, c0 : c0 + 128],
                      wt_sb[b : b + 34, 0:NODES],
                  )

              def chain(eng, vp, dp, gi, vcur3, g3, s0, s1, tagsuf=""):
                  """Value-tree levels k=5..0 on slot range [s0, s1)."""
                  sw = s1 - s0
                  for k in range(5, -1, -1):
                      m = 2 ** k
                      off = LEVEL_OFF[k]
                      vl = vcur3[:, 0:m, s0:s1] if k == 5 else vcur3[:, 0:m, :]
                      vr = vcur3[:, m : 2 * m, s0:s1] if k == 5 else vcur3[:, m : 2 * m, :]
                      dt = dp.tile([128, m, sw], bf, tag=f"dtmp{k}{tagsuf}")
                      eng.tensor_sub(dt[:], vl, vr)
                      gk = g3[:, off : off + m, s0:s1]
                      if k > 0:
                          vn = vp.tile([128, m, sw], bf, tag=f"v{k}{tagsuf}")
                          eng.tensor_mul(vn[:], gk, dt[:])
                          eng.tensor_add(vn[:], vn[:], vr)
                          vcur3 = vn[:]
                      else:
                          vo = out_sb[:, gi * slots + s0 : gi * slots + s1]
                          eng.tensor_mul(vo, gk, dt[:])
                          eng.tensor_add(vo, vo, vr)

              # greedy queued-time balance across DVE / Pool (ns of work
              # already assigned to each engine; costs from the cost model)
              qt = {"d": 0.0, "p": 0.0}
              W_D = (2 * 128 + 3 * 64) * pw * 0.5208 + 5 * 60  # wave init+k6
              W_P = (2 * 128 + 3 * 64) * pw * 0.8333
              C_D = 3 * 63 * slots * 0.5208 + 18 * 60          # k<=5 chain
              C_P = 3 * 63 * slots * 0.8333

              def pick(dc, pc):
                  if qt["d"] + dc <= qt["p"] + pc:
                      qt["d"] += dc
                      return nc.vector, vpool, dpool
                  qt["p"] += pc
                  return nc.gpsimd, vpoolp, dpoolp

              for gi in range(groups):
                  # g layout: [128, node, slot] — every tree slice is a flat
                  # contiguous range, keeping DVE in the bf16 2x perf mode.
                  g_t = gpool.tile([128, 256, slots], bf)
                  g3 = g_t[:]  # [128, node, slot]
                  db3 = dbt[:].rearrange("p (two node slot) -> p two node slot",
                                         two=2, node=128)
                  v = vpool.tile([128, 128, slots], bf, tag="v7")
                  for half in range(waves):
                      eng, vp, dp = pick(W_D, W_P)
                      zt = zps.tile([128, pw * 256], f32)
                      ztv = zt[:].rearrange("p (j c) -> p c j", c=256)
                      for j in range(pw):
                          g = gi * slots + half * pw + j
                          lhs, rhs = operands_of(g)
                          nc.tensor.matmul(
                              ztv[:, 0:NODES, j],
                              lhsT=lhs,
                              rhs=rhs[:, 0:NODES],
                              start=True,
                              stop=True,
                          )
                      hsl = slice(half * pw, (half + 1) * pw)
                      nc.scalar.activation(
                          out=g3[:, 0:NODES, hsl],
                          in_=ztv[:, 0:NODES, :],
                          func=mybir.ActivationFunctionType.Sigmoid,
                      )
                      # leaf init + k=6 for this wave's slots only
                      eng.tensor_mul(v[:, :, hsl], g3[:, 0:128, hsl],
                                     db3[:, 0, :, :])
                      eng.tensor_add(v[:, :, hsl], v[:, :, hsl],
                                     db3[:, 1, :, :])
                      dt6 = dp.tile([128, 64, pw], bf,
                                    tag="d6w" + ("p" if eng is nc.gpsimd else "d"))
                      eng.tensor_sub(dt6[:], v[:, 0:64, hsl],
                                     v[:, 64:128, hsl])
                      eng.tensor_mul(dt6[:], dt6[:],
                                     g3[:, 128:192, hsl])
                      eng.tensor_add(v[:, 0:64, hsl], dt6[:],
                                     v[:, 64:128, hsl])
                  # ---- remaining value-tree levels (k=5..0) ----
                  if split_last and gi == groups - 1:
                      qt["d"] += C_D / 2
                      qt["p"] += C_P / 2
                      chain(nc.vector, vpool, dpool, gi, v[:], g3, 0, slots // 2)
                      chain(nc.gpsimd, vpoolp, dpoolp, gi, v[:], g3,
                            slots // 2, slots, tagsuf="s")
                  else:
                      eng, vp, dp = pick(C_D, C_P)
                      chain(eng, vp, dp, gi, v[:], g3, 0, slots)

              step = max(1, groups // osplit)
              for g0 in range(0, groups, step):
                  c0, c1 = g0 * slots, min((g0 + step) * slots, tiles)
                  nc.sync.dma_start(out=out_dram[:, c0:c1], in_=out_sb[:, c0:c1])
    return nc


_CACHE = {}


def _get_nc(rows, slots, chunk):
    key = (rows, slots, chunk)
    if key not in _CACHE:
        nc = build_nc(rows, slots, chunk)
        if not nc.is_finalized():
            nc.finalize()
        _CACHE[key] = nc
    return _CACHE[key]


def host_xt(xc):
    """Host-side transpose of one core's x slice into the device layout:
    partition 64*s + f, col j == x[2j+s, f]; ones on partitions 32:34/96:98."""
    rows = xc.shape[0]
    xt = np.zeros((128, rows // 2), BF16)
    xb = xc.astype(BF16)
    xt[0:32] = xb[0::2].T
    xt[64:96] = xb[1::2].T
    xt[32:34] = 1
    xt[96:98] = 1
    return xt


def run_device(x, wt, db, slots, chunk, n_cores=N_CORES, trace=False):
    rows = x.shape[0] // n_cores
    nc = _get_nc(rows, slots, chunk)
    in_maps = [
        {
            "xt": host_xt(x[i * rows : (i + 1) * rows]),
            "wt": wt,
            "db": db,
        }
        for i in range(n_cores)
    ]
    res = run_bass_kernel_spmd(nc, in_maps, list(range(n_cores)), trace=trace)
    perm = out_permutation(rows, chunk)
    out = np.empty((n_cores * rows, 1), np.float32)
    for i in range(n_cores):
        dev = res.results[i]["out"].astype(np.float32)  # [128, tiles]
        core_out = np.empty(rows, np.float32)
        core_out[perm.ravel()] = dev.ravel()
        out[i * rows : (i + 1) * rows, 0] = core_out
    return out, res


def kernel(**inputs):
    x = np.asarray(inputs["x"], np.float32).reshape(-1, F)
    slots, chunk = 32, 4096
    wt, db = host_prep(
        inputs["feature_importances"],
        inputs["feature_splits"],
        inputs["leaf_node_classes"],
        8,  # db replicated per sigmoid wave (pw slots), not per group
    )
    out, _ = run_device(x, wt, db, slots, chunk)
    return out
